# revision 1
# baseline (speedup 1.0000x reference)
"""CIELUV channel loss kernel for 8 TRN2 NeuronCores (Bass/Tile).

Math (reference):
  luv = CIELUV(rgb);  a = box15(luv(input));  b = box15(luv(target))
  loss = sum_c mean_{n,h,w}((a-b)^2)

Kernel reformulation (exact up to bf16/fp32 rounding):
  - box filter is linear  ->  a - b = box15(luv(in) - luv(tgt))
  - per-channel means share a denominator -> loss = (global sum of squares) / (N*H*W)
  - f(t)=cbrt(t) branch: P[t<0.008856] ~ 2e-5 for uniform inputs and the
    linear branch is the tangent of cbrt at the threshold, so f(t)=exp(ln(t)/3)
    everywhere (error contribution < 1e-4 relative).
  - With L = 1508 fy - 208 (= 13 l): u = L*(fx-fy), v = L*(fy-fz);
    d_l = 116*dfy, the 116^2 is folded into the final combine.
  - 2D box filter = two banded matmuls on the PE (Band[h,i]=1 iff |h-i|<=7)
    applied to the three diff planes (dfy, du, dv); zero padding == band
    clipping at the borders.
  - sum(z^2) via bn_stats/bn_aggr (psum allows only one read operand).

Sharding: pure data parallel over N=16 -> 2 images per core; each core emits
[128,1] fp32 partial sums of squares; host reduces and divides.
"""

import numpy as np
import ml_dtypes
from contextlib import ExitStack

import concourse.bacc as bacc
import concourse.mybir as mybir
import concourse.tile as tile
from concourse.bass_utils import run_bass_kernel_spmd

F32 = mybir.dt.float32
F32R = mybir.dt.float32r
BF16 = mybir.dt.bfloat16
AF = mybir.ActivationFunctionType
OP = mybir.AluOpType

N_CORES = 8
IMGS_PER_CORE = 2
H = 512
W = 512
PATCH = 15
PAD = PATCH // 2  # 7
RB = H // 128  # 4 row blocks of 128

# Color matrix with white point folded in; plane order (x, z, y).
_M3 = [
    [0.4124564 / 0.95047, 0.3575761 / 0.95047, 0.1804375 / 0.95047],  # x
    [0.0193339 / 1.08883, 0.1191920 / 1.08883, 0.9503041 / 1.08883],  # z
    [0.2126729, 0.7151522, 0.0721750],                                # y
]

_CACHE = {}


def _build_nc():
    if "nc" in _CACHE:
        return _CACHE["nc"]

    nc = bacc.Bacc(None, target_bir_lowering=False, debug=False)
    inp = nc.dram_tensor("inp", [IMGS_PER_CORE, 3, H, W], F32R, kind="ExternalInput")
    tgt = nc.dram_tensor("tgt", [IMGS_PER_CORE, 3, H, W], F32R, kind="ExternalInput")
    band_d = nc.dram_tensor("band", [RB, 128, H], BF16, kind="ExternalInput")
    ident_d = nc.dram_tensor("ident", [9, 128, 128], F32R, kind="ExternalInput")
    acc_d = nc.dram_tensor("acc", [128, 1], F32, kind="ExternalOutput")

    with tile.TileContext(nc) as tc, ExitStack() as ctx:
        consts = ctx.enter_context(tc.tile_pool(name="consts", bufs=1))
        rgb_pool = ctx.enter_context(tc.tile_pool(name="rgb", bufs=3))
        lnt_pool = ctx.enter_context(tc.tile_pool(name="lnt", bufs=1))
        f_pool = ctx.enter_context(tc.tile_pool(name="fp", bufs=1))
        luv_pool = ctx.enter_context(tc.tile_pool(name="luv", bufs=1))
        feat_pool = ctx.enter_context(tc.tile_pool(name="feat", bufs=2))
        vt_pool = ctx.enter_context(tc.tile_pool(name="vt", bufs=1))
        sq_pool = ctx.enter_context(tc.tile_pool(name="sq", bufs=1))
        acc_pool = ctx.enter_context(tc.tile_pool(name="accp", bufs=2))
        xyz_psum = ctx.enter_context(tc.tile_pool(name="xyzp", bufs=2, space="PSUM"))
        filt_psum = ctx.enter_context(tc.tile_pool(name="filtp", bufs=2, space="PSUM"))

        band_sb = consts.tile([128, RB, H], BF16)
        nc.sync.dma_start(out=band_sb, in_=band_d[:].rearrange("j p i -> p j i"))
        ident_sb = consts.tile([128, 9, 128], F32R)
        nc.sync.dma_start(out=ident_sb, in_=ident_d[:].rearrange("k p m -> p k m"))

        def stage_xyz_ln(img, t):
            """XYZ matmuls + Ln for one tensor of one image -> lnt tile."""
            src = (inp, tgt)[t]
            lnt = lnt_pool.tile([128, 3, RB, W], F32, tag=f"lnt{t}",
                                name=f"lnt{t}")
            for rb in range(RB):
                rgb = rgb_pool.tile([128, 3, W], F32R, tag="rgb", name="rgb")
                nc.sync.dma_start(
                    out=rgb,
                    in_=src[img, :, rb * 128:(rb + 1) * 128, :].rearrange(
                        "c p w -> p c w"),
                )
                xyz = xyz_psum.tile([128, 3, W], F32, tag="xyz", name="xyz")
                for oc in range(3):
                    for ic in range(3):
                        nc.tensor.matmul(
                            xyz[:, oc, :],
                            lhsT=ident_sb[:, 3 * oc + ic, :],
                            rhs=rgb[:, ic, :],
                            start=(ic == 0),
                            stop=(ic == 2),
                        )
                # all Ln ops back-to-back on ACT -> one table set load
                nc.scalar.activation(lnt[:, :, rb, :], xyz[:], AF.Ln)
            return lnt

        def make_features(img, lnts):
            """Returns (DFY, DU, DV) diff planes [128, RB*W] bf16 for img."""
            # One Exp per tensor over the whole image: f = exp(ln/3), bf16
            fs = []
            for t in range(2):
                f = f_pool.tile([128, 3, RB, W], BF16, tag=f"f{t}", name=f"f{t}")
                fs.append(f)
                nc.scalar.activation(f[:], lnts[t][:], AF.Exp, scale=1.0 / 3.0)
            # LUV diff planes; fx=plane0, fz=plane1, fy=plane2 (x,z,y order).
            # All APs flattened to [128, RB*W] so DVE picks its 2x bf16 mode.
            uvs = []
            for t in range(2):
                f2 = fs[t].rearrange("p c a b -> p c (a b)")
                fy = f2[:, 2]    # [128, RB*W] contiguous
                L = luv_pool.tile([128, RB * W], BF16, tag="L", name="L")
                nc.gpsimd.tensor_scalar(L[:], fy, 1508.0, -208.0, OP.mult,
                                        OP.add)
                g1 = luv_pool.tile([128, RB * W], BF16, tag="g1", name="g1")
                nc.vector.tensor_sub(g1[:], f2[:, 0], fy)
                g2 = luv_pool.tile([128, RB * W], BF16, tag="g2", name="g2")
                nc.vector.tensor_sub(g2[:], fy, f2[:, 1])
                U = luv_pool.tile([128, RB * W], BF16, tag=f"U{t}", name=f"U{t}")
                nc.vector.tensor_mul(U[:], L[:], g1[:])
                V = luv_pool.tile([128, RB * W], BF16, tag=f"V{t}", name=f"V{t}")
                nc.vector.tensor_mul(V[:], L[:], g2[:])
                uvs.append((U, V))
            f0 = fs[0].rearrange("p c a b -> p c (a b)")
            f1 = fs[1].rearrange("p c a b -> p c (a b)")
            DFY = feat_pool.tile([128, RB * W], BF16, tag="DFY", name="DFY")
            nc.vector.tensor_sub(DFY[:], f0[:, 2], f1[:, 2])
            DU = feat_pool.tile([128, RB * W], BF16, tag="DU", name="DU")
            nc.vector.tensor_sub(DU[:], uvs[0][0][:], uvs[1][0][:])
            DV = feat_pool.tile([128, RB * W], BF16, tag="DV", name="DV")
            nc.vector.tensor_sub(DV[:], uvs[0][1][:], uvs[1][1][:])
            return (DFY, DU, DV)

        def banded_pass(psum, F):
            """psum[:, i] += sum_h F[h (partition), jb, m-block] * Band[h, i].
            F free dim already sliced to the 128-wide lhsT M block.
            Single start marks the whole 2KB psum bank pending-zero; every
            byte's first writer overwrites, later writers accumulate. Order
            pinned with explicit deps (Tile reorders accumulates)."""
            accs = []
            for jb in range(RB):
                accs.append((
                    psum[:, 128 * jb:128 * (jb + 1)],
                    F[:, jb],
                    band_sb[:, jb, 128 * jb:128 * (jb + 1)],
                ))
            # corner A: h in last 7 rows of chunk jb-1 (K base must be 0/32/64;
            # band rows 64..120 are zero there). corner B: first 7 of jb+1.
            for jb in range(1, RB):
                accs.append((
                    psum[:, 128 * jb:128 * jb + PAD],
                    F[64:128, jb - 1],
                    band_sb[64:128, jb - 1, 128 * jb:128 * jb + PAD],
                ))
            for jb in range(RB - 1):
                accs.append((
                    psum[:, 128 * jb + 121:128 * (jb + 1)],
                    F[0:7, jb + 1],
                    band_sb[0:7, jb + 1, 128 * jb + 121:128 * (jb + 1)],
                ))
            start_mm = None
            for i, (out, lhsT, rhs) in enumerate(accs):
                mm = nc.tensor.matmul(out, lhsT=lhsT, rhs=rhs, start=(i == 0),
                                      stop=(i == len(accs) - 1),
                                      skip_group_check=True)
                if i == 0:
                    start_mm = mm
                else:
                    tile.add_dep_helper(mm.ins, start_mm.ins, sync=False,
                                        reason="psum accumulate after start")

        n_ztiles = IMGS_PER_CORE * RB
        stats = [sq_pool.tile([128, n_ztiles, 6], F32, tag=f"stats{c}",
                              name=f"stats{c}") for c in range(3)]

        def filt_p1(img, ch, F):
            Fv = F.rearrange("p (a b) -> p a b", a=RB)
            VT = vt_pool.tile([128, RB, H], BF16, tag=f"VT{img}{ch}",
                              name=f"VT{img}{ch}")
            for jw in range(RB):
                p1 = filt_psum.tile([128, H], F32, tag="filt", name="p1")
                banded_pass(p1, Fv[:, :, 128 * jw:128 * (jw + 1)])
                nc.vector.tensor_copy(VT[:, jw, :], p1[:])
            return VT

        def filt_p2(img, ch, VT):
            for m in range(RB):
                p2 = filt_psum.tile([128, H], F32, tag="filt", name="p2")
                banded_pass(p2, VT[:, :, 128 * m:128 * (m + 1)])
                nc.vector.bn_stats(stats[ch][:, img * RB + m, :], p2[:])

        # Interleave image 1's feature pipeline into image 0's filter phase so
        # the PE stream stays dense (HAM stays warm, stalls overlapped).
        lnts0 = [stage_xyz_ln(0, 0), stage_xyz_ln(0, 1)]
        feats0 = make_features(0, lnts0)
        vt00 = filt_p1(0, 0, feats0[0])
        lnts1_0 = stage_xyz_ln(1, 0)
        vt01 = filt_p1(0, 1, feats0[1])
        lnts1_1 = stage_xyz_ln(1, 1)
        vt02 = filt_p1(0, 2, feats0[2])
        feats1 = make_features(1, [lnts1_0, lnts1_1])
        vt10 = filt_p1(1, 0, feats1[0])
        filt_p2(0, 0, vt00)
        vt11 = filt_p1(1, 1, feats1[1])
        filt_p2(0, 1, vt01)
        vt12 = filt_p1(1, 2, feats1[2])
        filt_p2(0, 2, vt02)
        filt_p2(1, 0, vt10)
        filt_p2(1, 1, vt11)
        filt_p2(1, 2, vt12)

        # per-channel: n*(var + mean^2); l scaled by 116^2; sum channels
        nvals = float(n_ztiles * W)
        acc = None
        for ch in range(3):
            mv = acc_pool.tile([128, 2], F32, tag="mv", name="mv")
            nc.vector.bn_aggr(mv[:], stats[ch][:])
            m2 = acc_pool.tile([128, 1], F32, tag="m2", name="m2")
            nc.vector.tensor_tensor(m2[:], mv[:, 0:1], mv[:, 0:1], OP.mult)
            s = acc_pool.tile([128, 1], F32, tag=f"s{ch}", name=f"s{ch}")
            nc.vector.tensor_tensor(s[:], m2[:], mv[:, 1:2], OP.add)
            w = nvals * (116.0 * 116.0 if ch == 0 else 1.0)
            acc_new = acc_pool.tile([128, 1], F32, tag=f"acc{ch}",
                                    name=f"acc{ch}")
            if acc is None:
                nc.vector.tensor_scalar_mul(acc_new[:], s[:], w)
            else:
                nc.vector.scalar_tensor_tensor(acc_new[:], s[:], w, acc[:],
                                               OP.mult, OP.add)
            acc = acc_new

        nc.sync.dma_start(out=acc_d[:], in_=acc[:])

    nc.compile()
    _CACHE["nc"] = nc
    return nc


def _consts_np():
    band = np.zeros((H, H), np.float32)
    i = np.arange(H)
    for dd in range(-PAD, PAD + 1):
        j = i + dd
        m = (j >= 0) & (j < H)
        band[i[m], j[m]] = 1.0
    band = band.reshape(RB, 128, H).astype(ml_dtypes.bfloat16)

    ident = np.zeros((9, 128, 128), np.float32)
    for oc in range(3):
        for ic in range(3):
            np.fill_diagonal(ident[3 * oc + ic], _M3[oc][ic])
    return band, ident


def _run(input, target, trace=False, **kw):
    nc = _build_nc()
    band, ident = _consts_np()
    in_maps = []
    for c in range(N_CORES):
        s = slice(c * IMGS_PER_CORE, (c + 1) * IMGS_PER_CORE)
        in_maps.append({
            "inp": np.ascontiguousarray(input[s]),
            "tgt": np.ascontiguousarray(target[s]),
            "band": band,
            "ident": ident,
        })
    return run_bass_kernel_spmd(nc, in_maps, core_ids=list(range(N_CORES)),
                                trace=trace, **kw)


def kernel(input, target, patch_size):
    assert int(np.asarray(patch_size)) == PATCH
    input = np.asarray(input, dtype=np.float32)
    target = np.asarray(target, dtype=np.float32)
    res = _run(input, target)
    total = 0.0
    for r in res.results:
        total += float(np.asarray(r["acc"]).astype(np.float64).sum())
    n = input.shape[0]
    return np.asarray(total / (n * H * W), dtype=np.float32)



# revision 7
# speedup vs baseline: 1.1152x; 1.1152x over previous
"""CIELUV channel loss kernel for 8 TRN2 NeuronCores (Bass/Tile).

Math (reference):
  luv = CIELUV(rgb);  a = box15(luv(input));  b = box15(luv(target))
  loss = sum_c mean_{n,h,w}((a-b)^2)

Kernel reformulation (exact up to bf16/fp32 rounding):
  - box filter is linear  ->  a - b = box15(luv(in) - luv(tgt))
  - per-channel means share a denominator -> loss = (global sum of squares) / (N*H*W)
  - f(t)=cbrt(t) branch: P[t<0.008856] ~ 2e-5 for uniform inputs and the
    linear branch is the tangent of cbrt at the threshold, so f(t)=exp(ln(t)/3)
    everywhere (error contribution < 1e-4 relative).
  - With L = 1508 fy - 208 (= 13 l): u = L*(fx-fy), v = L*(fy-fz);
    d_l = 116*dfy, the 116^2 is folded into the final combine.
  - 2D box filter = two banded matmuls on the PE (Band[h,i]=1 iff |h-i|<=7)
    applied to the three diff planes (dfy, du, dv); zero padding == band
    clipping at the borders.
  - Each banded pass is 4 matmuls per 128-wide slab: block jb of the band
    contributes to outputs i in [128*jb-7, 128*(jb+1)+7) only, so the main
    matmul's psum range is simply extended by the corner overlap (ranges of
    consecutive accumulates overlap; first writer of a byte overwrites,
    later writers accumulate). No separate corner matmuls.
  - sum(z^2) via bn_stats/bn_aggr (psum allows only one read operand).
  - Ln and Exp both live in the 'natural_log_exp_and_others' ACT table; the
    cached table dict is narrowed during compile so the table-load inserter
    picks that set once instead of thrashing natural_log/exp_and_others.

Sharding: pure data parallel over N=16 -> 2 images per core; each core emits
[128,1] fp32 partial sums of squares; host reduces and divides.
"""

import numpy as np
import ml_dtypes
from contextlib import ExitStack

import concourse.bacc as bacc
import concourse.mybir as mybir
import concourse.tile as tile
from concourse.bass_utils import run_bass_kernel_spmd

F32 = mybir.dt.float32
F32R = mybir.dt.float32r
BF16 = mybir.dt.bfloat16
AF = mybir.ActivationFunctionType
OP = mybir.AluOpType

N_CORES = 8
IMGS_PER_CORE = 2
H = 512
W = 512
PATCH = 15
PAD = PATCH // 2  # 7
RB = H // 128  # 4 row blocks of 128
# extended psum ranges per band block: block jb touches outputs
# [128*jb-7, 128*(jb+1)+7) clipped to [0, 512)
LO = [max(0, 128 * jb - PAD) for jb in range(RB)]
HI = [min(H, 128 * (jb + 1) + PAD) for jb in range(RB)]

# Color matrix with white point folded in; plane order (x, z, y).
_M3 = [
    [0.4124564 / 0.95047, 0.3575761 / 0.95047, 0.1804375 / 0.95047],  # x
    [0.0193339 / 1.08883, 0.1191920 / 1.08883, 0.9503041 / 1.08883],  # z
    [0.2126729, 0.7151522, 0.0721750],                                # y
]

_CACHE = {}
_COMBINED_TABLE = "natural_log_exp_and_others"


class _ActTableNarrow:
    """Narrow the cached ACT-table sets so Ln/Exp resolve only to the
    combined table; restores the shared dict on exit."""

    def __init__(self, arch):
        from concourse.hw_specs import get_activation_tables
        self.tabs = get_activation_tables(arch)

    def __enter__(self):
        self.saved = {k: set(v) for k, v in self.tabs.items()}
        assert _COMBINED_TABLE in self.tabs
        assert AF.Ln in self.tabs[_COMBINED_TABLE]
        assert AF.Exp in self.tabs[_COMBINED_TABLE]
        for name, s in self.tabs.items():
            if name != _COMBINED_TABLE:
                s.discard(AF.Ln)
                s.discard(AF.Exp)
        return self

    def __exit__(self, *exc):
        for name, s in self.tabs.items():
            s.clear()
            s.update(self.saved[name])
        return False


def _build_nc():
    if "nc" in _CACHE:
        return _CACHE["nc"]

    nc = bacc.Bacc(None, target_bir_lowering=False, debug=False)
    inp = nc.dram_tensor("inp", [IMGS_PER_CORE, 3, H, W], F32R, kind="ExternalInput")
    tgt = nc.dram_tensor("tgt", [IMGS_PER_CORE, 3, H, W], F32R, kind="ExternalInput")
    band_d = nc.dram_tensor("band", [RB, 128, H], BF16, kind="ExternalInput")
    ident_d = nc.dram_tensor("ident", [9, 128, 128], F32R, kind="ExternalInput")
    acc_d = nc.dram_tensor("acc", [128, 1], F32, kind="ExternalOutput")

    with tile.TileContext(nc) as tc, ExitStack() as ctx:
        consts = ctx.enter_context(tc.tile_pool(name="consts", bufs=1))
        rgb_pool = ctx.enter_context(tc.tile_pool(name="rgb", bufs=3))
        lnt_pool = ctx.enter_context(tc.tile_pool(name="lnt", bufs=1))
        f_pool = ctx.enter_context(tc.tile_pool(name="fp", bufs=1))
        luv_pool = ctx.enter_context(tc.tile_pool(name="luv", bufs=1))
        feat_pool = ctx.enter_context(tc.tile_pool(name="feat", bufs=1))
        vt_pool = ctx.enter_context(tc.tile_pool(name="vt", bufs=1))
        sq_pool = ctx.enter_context(tc.tile_pool(name="sq", bufs=1))
        acc_pool = ctx.enter_context(tc.tile_pool(name="accp", bufs=2))
        xyz_psum = ctx.enter_context(tc.tile_pool(name="xyzp", bufs=2, space="PSUM"))
        filt_psum = ctx.enter_context(tc.tile_pool(name="filtp", bufs=2, space="PSUM"))

        band_sb = consts.tile([128, RB, H], BF16)
        nc.sync.dma_start(out=band_sb, in_=band_d[:].rearrange("j p i -> p j i"))
        ident_sb = consts.tile([128, 9, 128], F32R)
        nc.sync.dma_start(out=ident_sb, in_=ident_d[:].rearrange("k p m -> p k m"))

        def xyz_ln(it):
            """XYZ matmuls + Ln for image-tensor it -> bf16 lnt tile."""
            img, t = divmod(it, 2)
            src = (inp, tgt)[t]
            lnt = lnt_pool.tile([128, 3, RB, W], BF16, tag=f"lnt{t}",
                                name=f"lnt{it}")
            for rb in range(RB):
                rgb = rgb_pool.tile([128, 3, W], F32R, tag="rgb", name="rgb")
                nc.sync.dma_start(
                    out=rgb,
                    in_=src[img, :, rb * 128:(rb + 1) * 128, :].rearrange(
                        "c p w -> p c w"),
                )
                xyz = xyz_psum.tile([128, 3, W], F32, tag="xyz", name="xyz")
                for oc in range(3):
                    for ic in range(3):
                        nc.tensor.matmul(
                            xyz[:, oc, :],
                            lhsT=ident_sb[:, 3 * oc + ic, :],
                            rhs=rgb[:, ic, :],
                            start=(ic == 0),
                            stop=(ic == 2),
                        )
                nc.scalar.activation(lnt[:, :, rb, :], xyz[:], AF.Ln)
            return lnt

        def exp_f(it, lnt):
            """f = exp(lnt/3) over the whole image-tensor, bf16."""
            f = f_pool.tile([128, 3, RB, W], BF16, tag=f"f{it % 2}",
                            name=f"f{it}")
            nc.scalar.activation(f[:], lnt[:], AF.Exp, scale=1.0 / 3.0)
            return f

        def feat_dfy(img, f_a, f_b):
            """DFY plane + L factors for an image pair."""
            fa = f_a.rearrange("p c a b -> p c (a b)")
            fb = f_b.rearrange("p c a b -> p c (a b)")
            DFY = feat_pool.tile([128, RB * W], BF16, tag=f"DFY{img}",
                                 name=f"DFY{img}")
            nc.vector.tensor_sub(DFY[:], fa[:, 2], fb[:, 2])
            Ls = []
            for t, f2 in ((0, fa), (1, fb)):
                L = luv_pool.tile([128, RB * W], BF16, tag=f"L{t}",
                                  name=f"L{img}{t}")
                nc.gpsimd.tensor_scalar(L[:], f2[:, 2], 1508.0, -208.0,
                                        OP.mult, OP.add)
                Ls.append(L)
            return DFY, Ls

        def feat_du(img, f_a, f_b, Ls):
            fa = f_a.rearrange("p c a b -> p c (a b)")
            fb = f_b.rearrange("p c a b -> p c (a b)")
            Us = []
            for t, f2 in ((0, fa), (1, fb)):
                g1 = luv_pool.tile([128, RB * W], BF16, tag=f"g1{t}",
                                   name=f"g1{img}{t}")
                nc.vector.tensor_sub(g1[:], f2[:, 0], f2[:, 2])
                U = luv_pool.tile([128, RB * W], BF16, tag=f"U{t}",
                                  name=f"U{img}{t}")
                nc.gpsimd.tensor_mul(U[:], Ls[t][:], g1[:])
                Us.append(U)
            DU = feat_pool.tile([128, RB * W], BF16, tag=f"DU{img}",
                                name=f"DU{img}")
            nc.vector.tensor_sub(DU[:], Us[0][:], Us[1][:])
            return DU

        def feat_dv(img, f_a, f_b, Ls):
            fa = f_a.rearrange("p c a b -> p c (a b)")
            fb = f_b.rearrange("p c a b -> p c (a b)")
            Vs = []
            for t, f2 in ((0, fa), (1, fb)):
                g2 = luv_pool.tile([128, RB * W], BF16, tag=f"g2{t}",
                                   name=f"g2{img}{t}")
                nc.vector.tensor_sub(g2[:], f2[:, 2], f2[:, 1])
                V = luv_pool.tile([128, RB * W], BF16, tag=f"V{t}",
                                  name=f"V{img}{t}")
                nc.gpsimd.tensor_mul(V[:], Ls[t][:], g2[:])
                Vs.append(V)
            DV = feat_pool.tile([128, RB * W], BF16, tag=f"DV{img}",
                                name=f"DV{img}")
            nc.vector.tensor_sub(DV[:], Vs[0][:], Vs[1][:])
            return DV

        def banded_pass(ps, lhsT_of_jb):
            """ps[:, i] += sum_h lhsT[h, m] * Band[h, i], 4 extended-range
            accumulating matmuls; order pinned (Tile reorders accumulates)."""
            prev = None
            for jb in range(RB):
                mm = nc.tensor.matmul(
                    ps[:, LO[jb]:HI[jb]],
                    lhsT=lhsT_of_jb(jb),
                    rhs=band_sb[:, jb, LO[jb]:HI[jb]],
                    start=(jb == 0),
                    stop=(jb == RB - 1),
                    skip_group_check=True,
                )
                if prev is not None:
                    tile.add_dep_helper(mm.ins, prev.ins, sync=False,
                                        reason="psum accumulate order")
                prev = mm

        n_ztiles = IMGS_PER_CORE * RB
        stats = [sq_pool.tile([128, n_ztiles, 6], F32, tag=f"stats{c}",
                              name=f"stats{c}") for c in range(3)]

        def filt_p1(img, ch, F):
            """Column pass: psum[w, h'] = sum_h F[h, w] Band[h, h']."""
            Fv = F.rearrange("p (a b) -> p a b", a=RB)
            VT = vt_pool.tile([128, RB, H], BF16, tag=f"VT{img}{ch}",
                              name=f"VT{img}{ch}")
            for jw in range(RB):
                p1 = filt_psum.tile([128, H], F32, tag="filt", name="p1")
                banded_pass(p1, lambda jb: Fv[:, jb, 128 * jw:128 * (jw + 1)])
                nc.vector.tensor_copy(VT[:, jw, :], p1[:])
            return VT

        def filt_p2(img, ch, VT):
            """Row pass + sum of squares via bn_stats."""
            for m in range(RB):
                p2 = filt_psum.tile([128, H], F32, tag="filt", name="p2")
                banded_pass(p2, lambda jw: VT[:, jw, 128 * m:128 * (m + 1)])
                nc.vector.bn_stats(stats[ch][:, img * RB + m, :], p2[:])

        # Software-pipelined emission. Per-engine queue order == emission
        # order, so filters of image 0 interleave with XYZ of image 1 on the
        # PE, and ACT alternates Ln/Exp (single shared table, no reloads).
        lnt0 = xyz_ln(0)
        lnt1 = xyz_ln(1)
        f0 = exp_f(0, lnt0)
        lnt2 = xyz_ln(2)
        f1 = exp_f(1, lnt1)
        dfy0, ls0 = feat_dfy(0, f0, f1)
        vt00 = filt_p1(0, 0, dfy0)
        lnt3 = xyz_ln(3)
        du0 = feat_du(0, f0, f1, ls0)
        vt01 = filt_p1(0, 1, du0)
        dv0 = feat_dv(0, f0, f1, ls0)
        vt02 = filt_p1(0, 2, dv0)
        f2 = exp_f(2, lnt2)
        f3 = exp_f(3, lnt3)
        filt_p2(0, 0, vt00)
        dfy1, ls1 = feat_dfy(1, f2, f3)
        filt_p2(0, 1, vt01)
        du1 = feat_du(1, f2, f3, ls1)
        filt_p2(0, 2, vt02)
        dv1 = feat_dv(1, f2, f3, ls1)
        vt10 = filt_p1(1, 0, dfy1)
        vt11 = filt_p1(1, 1, du1)
        vt12 = filt_p1(1, 2, dv1)
        filt_p2(1, 0, vt10)
        filt_p2(1, 1, vt11)
        filt_p2(1, 2, vt12)

        # per-channel: n*(var + mean^2); l scaled by 116^2; sum channels
        nvals = float(n_ztiles * W)
        acc = None
        for ch in range(3):
            mv = acc_pool.tile([128, 2], F32, tag="mv", name="mv")
            nc.vector.bn_aggr(mv[:], stats[ch][:])
            m2 = acc_pool.tile([128, 1], F32, tag="m2", name="m2")
            nc.vector.tensor_tensor(m2[:], mv[:, 0:1], mv[:, 0:1], OP.mult)
            s = acc_pool.tile([128, 1], F32, tag=f"s{ch}", name=f"s{ch}")
            nc.vector.tensor_tensor(s[:], m2[:], mv[:, 1:2], OP.add)
            w = nvals * (116.0 * 116.0 if ch == 0 else 1.0)
            acc_new = acc_pool.tile([128, 1], F32, tag=f"acc{ch}",
                                    name=f"acc{ch}")
            if acc is None:
                nc.vector.tensor_scalar_mul(acc_new[:], s[:], w)
            else:
                nc.vector.scalar_tensor_tensor(acc_new[:], s[:], w, acc[:],
                                               OP.mult, OP.add)
            acc = acc_new

        nc.sync.dma_start(out=acc_d[:], in_=acc[:])

    with _ActTableNarrow(nc.m.arch):
        nc.compile()
    _CACHE["nc"] = nc
    return nc


def _consts_np():
    band = np.zeros((H, H), np.float32)
    i = np.arange(H)
    for dd in range(-PAD, PAD + 1):
        j = i + dd
        m = (j >= 0) & (j < H)
        band[i[m], j[m]] = 1.0
    band = band.reshape(RB, 128, H).astype(ml_dtypes.bfloat16)

    ident = np.zeros((9, 128, 128), np.float32)
    for oc in range(3):
        for ic in range(3):
            np.fill_diagonal(ident[3 * oc + ic], _M3[oc][ic])
    return band, ident


def _run(input, target, trace=False, **kw):
    nc = _build_nc()
    band, ident = _consts_np()
    in_maps = []
    for c in range(N_CORES):
        s = slice(c * IMGS_PER_CORE, (c + 1) * IMGS_PER_CORE)
        in_maps.append({
            "inp": np.ascontiguousarray(input[s]),
            "tgt": np.ascontiguousarray(target[s]),
            "band": band,
            "ident": ident,
        })
    return run_bass_kernel_spmd(nc, in_maps, core_ids=list(range(N_CORES)),
                                trace=trace, **kw)


def kernel(input, target, patch_size):
    assert int(np.asarray(patch_size)) == PATCH
    input = np.asarray(input, dtype=np.float32)
    target = np.asarray(target, dtype=np.float32)
    res = _run(input, target)
    total = 0.0
    for r in res.results:
        total += float(np.asarray(r["acc"]).astype(np.float64).sum())
    n = input.shape[0]
    return np.asarray(total / (n * H * W), dtype=np.float32)


# revision 10
# speedup vs baseline: 1.1929x; 1.0696x over previous
"""CIELUV channel loss kernel for 8 TRN2 NeuronCores (Bass/Tile).

Math (reference):
  luv = CIELUV(rgb);  a = box15(luv(input));  b = box15(luv(target))
  loss = sum_c mean_{n,h,w}((a-b)^2)

Kernel reformulation (exact up to bf16/fp32 rounding):
  - box filter is linear  ->  a - b = box15(luv(in) - luv(tgt))
  - per-channel means share a denominator -> loss = (global sum of squares) / (N*H*W)
  - f(t)=cbrt(t) branch: P[t<0.008856] ~ 2e-5 for uniform inputs and the
    linear branch is the tangent of cbrt at the threshold, so f(t)=exp(ln(t)/3)
    everywhere (error contribution < 1e-4 relative).
  - With L = 1508 fy - 208 (= 13 l): u = L*(fx-fy), v = L*(fy-fz);
    d_l = 116*dfy, the 116^2 is folded into the final combine.
  - 2D box filter = two banded matmuls on the PE (Band[h,i]=1 iff |h-i|<=7)
    applied per difference plane; zero padding == band clipping at borders.
  - The a-b difference of each plane is folded into the first banded pass:
    psum accumulates source_a x Band plus source_b x (-Band), so no explicit
    difference tensors are ever materialized.
  - Each banded pass needs 4 matmuls per source per 128-wide slab: band block
    jb only touches outputs [128*jb-7, 128*(jb+1)+7), so the psum ranges of
    consecutive accumulates simply overlap (first writer of a byte
    overwrites, later writers accumulate). No corner matmuls.
  - l/u planes: sum(z^2) via bn_stats/bn_aggr on DVE. v plane: Square
    activation with accum_out on the Scalar engine (idle after the Exps).
  - Ln and Exp both live in the 'natural_log_exp_and_others' ACT table; the
    cached table dict is narrowed during compile so the table-load inserter
    picks that set once instead of thrashing natural_log/exp_and_others.

Sharding: pure data parallel over N=16 -> 2 images per core; each core emits
[128,1] fp32 partial sums of squares; host reduces and divides.
"""

import numpy as np
import ml_dtypes
from contextlib import ExitStack

import concourse.bacc as bacc
import concourse.mybir as mybir
import concourse.tile as tile
from concourse.bass_utils import run_bass_kernel_spmd

F32 = mybir.dt.float32
F32R = mybir.dt.float32r
BF16 = mybir.dt.bfloat16
AF = mybir.ActivationFunctionType
OP = mybir.AluOpType

N_CORES = 8
IMGS_PER_CORE = 2
H = 512
W = 512
PATCH = 15
PAD = PATCH // 2  # 7
RB = H // 128  # 4 row blocks of 128
# extended psum ranges per band block: block jb touches outputs
# [128*jb-7, 128*(jb+1)+7) clipped to [0, 512)
LO = [max(0, 128 * jb - PAD) for jb in range(RB)]
HI = [min(H, 128 * (jb + 1) + PAD) for jb in range(RB)]

# Color matrix with white point folded in; plane order (x, z, y).
_M3 = [
    [0.4124564 / 0.95047, 0.3575761 / 0.95047, 0.1804375 / 0.95047],  # x
    [0.0193339 / 1.08883, 0.1191920 / 1.08883, 0.9503041 / 1.08883],  # z
    [0.2126729, 0.7151522, 0.0721750],                                # y
]

_CACHE = {}
_COMBINED_TABLE = "natural_log_exp_and_others"


class _ActTableNarrow:
    """Narrow the cached ACT-table sets so Ln/Exp resolve only to the
    combined table; restores the shared dict on exit."""

    def __init__(self, arch):
        from concourse.hw_specs import get_activation_tables
        self.tabs = get_activation_tables(arch)

    def __enter__(self):
        self.saved = {k: set(v) for k, v in self.tabs.items()}
        assert _COMBINED_TABLE in self.tabs
        assert AF.Ln in self.tabs[_COMBINED_TABLE]
        assert AF.Exp in self.tabs[_COMBINED_TABLE]
        for name, s in self.tabs.items():
            if name != _COMBINED_TABLE:
                s.discard(AF.Ln)
                s.discard(AF.Exp)
        return self

    def __exit__(self, *exc):
        for name, s in self.tabs.items():
            s.clear()
            s.update(self.saved[name])
        return False


def _build_nc():
    if "nc" in _CACHE:
        return _CACHE["nc"]

    nc = bacc.Bacc(None, target_bir_lowering=False, debug=False)
    inp = nc.dram_tensor("inp", [IMGS_PER_CORE, 3, H, W], F32R, kind="ExternalInput")
    tgt = nc.dram_tensor("tgt", [IMGS_PER_CORE, 3, H, W], F32R, kind="ExternalInput")
    band_d = nc.dram_tensor("band", [RB, 128, H], BF16, kind="ExternalInput")
    nband_d = nc.dram_tensor("nband", [RB, 128, H], BF16, kind="ExternalInput")
    ident_d = nc.dram_tensor("ident", [9, 128, 128], F32R, kind="ExternalInput")
    acc_d = nc.dram_tensor("acc", [128, 1], F32, kind="ExternalOutput")

    with tile.TileContext(nc) as tc, ExitStack() as ctx:
        consts = ctx.enter_context(tc.tile_pool(name="consts", bufs=1))
        rgb_pool = ctx.enter_context(tc.tile_pool(name="rgb", bufs=6))
        lnt_pool = ctx.enter_context(tc.tile_pool(name="lnt", bufs=1))
        f_pool = ctx.enter_context(tc.tile_pool(name="fp", bufs=1))
        luv_pool = ctx.enter_context(tc.tile_pool(name="luv", bufs=1))
        vt_pool = ctx.enter_context(tc.tile_pool(name="vt", bufs=1))
        sq_pool = ctx.enter_context(tc.tile_pool(name="sq", bufs=1))
        acc_pool = ctx.enter_context(tc.tile_pool(name="accp", bufs=2))
        xyz_psum = ctx.enter_context(tc.tile_pool(name="xyzp", bufs=2, space="PSUM"))
        filt_psum = ctx.enter_context(tc.tile_pool(name="filtp", bufs=2, space="PSUM"))

        band_sb = consts.tile([128, RB, H], BF16)
        nc.sync.dma_start(out=band_sb, in_=band_d[:].rearrange("j p i -> p j i"))
        nband_sb = consts.tile([128, RB, H], BF16)
        nc.sync.dma_start(out=nband_sb, in_=nband_d[:].rearrange("j p i -> p j i"))
        ident_sb = consts.tile([128, 9, 128], F32R)
        nc.sync.dma_start(out=ident_sb, in_=ident_d[:].rearrange("k p m -> p k m"))

        def xyz_ln(it):
            """XYZ matmuls + Ln for image-tensor it -> bf16 lnt tile."""
            img, t = divmod(it, 2)
            src = (inp, tgt)[t]
            lnt = lnt_pool.tile([128, 3, RB, W], BF16, tag=f"lnt{t}",
                                name=f"lnt{it}")
            for rb in range(RB):
                rgb = rgb_pool.tile([128, 3, W], F32R, tag="rgb", name="rgb")
                nc.sync.dma_start(
                    out=rgb,
                    in_=src[img, :, rb * 128:(rb + 1) * 128, :].rearrange(
                        "c p w -> p c w"),
                )
                xyz = xyz_psum.tile([128, 3, W], F32, tag="xyz", name="xyz")
                for oc in range(3):
                    for ic in range(3):
                        nc.tensor.matmul(
                            xyz[:, oc, :],
                            lhsT=ident_sb[:, 3 * oc + ic, :],
                            rhs=rgb[:, ic, :],
                            start=(ic == 0),
                            stop=(ic == 2),
                        )
                nc.scalar.activation(lnt[:, :, rb, :], xyz[:], AF.Ln)
            return lnt

        def exp_f(it, lnt):
            """f = exp(lnt/3) over the whole image-tensor, bf16."""
            f = f_pool.tile([128, 3, RB, W], BF16, tag=f"f{it % 2}",
                            name=f"f{it}")
            nc.scalar.activation(f[:], lnt[:], AF.Exp, scale=1.0 / 3.0)
            return f

        def plane(f):
            return f.rearrange("p c a b -> p c (a b)")

        def mk_L(eng, img, t, f):
            L = luv_pool.tile([128, RB * W], BF16, tag=f"L{img}{t}",
                              name=f"L{img}{t}")
            eng.tensor_scalar(L[:], plane(f)[:, 2], 1508.0, -208.0,
                              OP.mult, OP.add)
            return L

        def mk_g1(eng, img, t, f):
            g = luv_pool.tile([128, RB * W], BF16, tag=f"g1{img}{t}",
                              name=f"g1{img}{t}")
            eng.tensor_sub(g[:], plane(f)[:, 0], plane(f)[:, 2])
            return g

        def mk_g2(eng, img, t, f):
            g = luv_pool.tile([128, RB * W], BF16, tag=f"g2{img}{t}",
                              name=f"g2{img}{t}")
            eng.tensor_sub(g[:], plane(f)[:, 2], plane(f)[:, 1])
            return g

        def mk_mul(img, t, nm, L, g):
            o = luv_pool.tile([128, RB * W], BF16, tag=f"{nm}{img}{t}",
                              name=f"{nm}{img}{t}")
            nc.vector.tensor_mul(o[:], L[:], g[:])
            return o

        n_ztiles = IMGS_PER_CORE * RB
        stats = [sq_pool.tile([128, n_ztiles, 6], F32, tag=f"stats{c}",
                              name=f"stats{c}") for c in range(2)]
        sq = sq_pool.tile([128, n_ztiles], F32, tag="sq", name="sq")
        scratch = sq_pool.tile([128, H], BF16, tag="scratch", name="scratch")

        def banded_pass(ps, sources):
            """ps[:, i] += sum_{src,h} lhsT_src[h, m] * (+-Band)[h, i].
            sources: list of (lhsT_of_jb, band_tile). Order pinned."""
            prev = None
            first = True
            n_src = len(sources)
            for si, (lhsT_of_jb, bnd) in enumerate(sources):
                for jb in range(RB):
                    mm = nc.tensor.matmul(
                        ps[:, LO[jb]:HI[jb]],
                        lhsT=lhsT_of_jb(jb),
                        rhs=bnd[:, jb, LO[jb]:HI[jb]],
                        start=first,
                        stop=(si == n_src - 1 and jb == RB - 1),
                        skip_group_check=True,
                    )
                    first = False
                    if prev is not None:
                        tile.add_dep_helper(mm.ins, prev.ins, sync=False,
                                            reason="psum accumulate order")
                    prev = mm

        def filt_p1(img, ch, src_a, src_b, cast_eng):
            """Column pass with the a-b diff folded in:
            psum[w, h'] = sum_h (a[h,w]-b[h,w]) Band[h,h']."""
            VT = vt_pool.tile([128, RB, H], BF16, tag=f"VT{ch}",
                              name=f"VT{img}{ch}")
            for jw in range(RB):
                p1 = filt_psum.tile([128, H], F32, tag="filt", name="p1")
                banded_pass(p1, [
                    (lambda jb: src_a[:, jb, 128 * jw:128 * (jw + 1)], band_sb),
                    (lambda jb: src_b[:, jb, 128 * jw:128 * (jw + 1)], nband_sb),
                ])
                if cast_eng == "act":
                    nc.scalar.copy(VT[:, jw, :], p1[:])
                else:
                    nc.vector.tensor_copy(VT[:, jw, :], p1[:])
            return VT

        def filt_p2(img, ch, VT):
            """Row pass + sum of squares (DVE bn_stats for l/u, ACT Square
            accumulator for v)."""
            for m in range(RB):
                p2 = filt_psum.tile([128, H], F32, tag="filt", name="p2")
                banded_pass(p2, [
                    (lambda jw: VT[:, jw, 128 * m:128 * (m + 1)], band_sb),
                ])
                k = img * RB + m
                if ch < 2:
                    nc.vector.bn_stats(stats[ch][:, k, :], p2[:])
                else:
                    nc.scalar.activation(scratch[:], p2[:], AF.Square,
                                         accum_out=sq[:, k:k + 1])

        def rb_view(t):
            return t[:].rearrange("p (a b) -> p a b", a=RB)

        # ---- software-pipelined emission (queue order == emission order) ---
        lnt0 = xyz_ln(0)
        lnt1 = xyz_ln(1)
        f0 = exp_f(0, lnt0)
        L0a = mk_L(nc.gpsimd, 0, 0, f0)
        g1a0 = mk_g1(nc.vector, 0, 0, f0)
        lnt2 = xyz_ln(2)
        f1 = exp_f(1, lnt1)
        L0b = mk_L(nc.gpsimd, 0, 1, f1)
        g1b0 = mk_g1(nc.vector, 0, 1, f1)
        g2a0 = mk_g2(nc.gpsimd, 0, 0, f0)
        g2b0 = mk_g2(nc.gpsimd, 0, 1, f1)
        Ua0 = mk_mul(0, 0, "U", L0a, g1a0)
        Ub0 = mk_mul(0, 1, "U", L0b, g1b0)
        fy0 = f0[:, 2]
        fy1 = f1[:, 2]
        vt_l0 = filt_p1(0, 0, fy0, fy1, "dve")
        lnt3 = xyz_ln(3)
        Va0 = mk_mul(0, 0, "V", L0a, g2a0)
        Vb0 = mk_mul(0, 1, "V", L0b, g2b0)
        vt_u0 = filt_p1(0, 1, rb_view(Ua0), rb_view(Ub0), "dve")
        f2 = exp_f(2, lnt2)
        vt_v0 = filt_p1(0, 2, rb_view(Va0), rb_view(Vb0), "act")
        f3 = exp_f(3, lnt3)
        # prerun img1 a-side while img0 filters drain
        L1a = mk_L(nc.vector, 1, 0, f2)
        g1a1 = mk_g1(nc.vector, 1, 0, f2)
        g2a1 = mk_g2(nc.gpsimd, 1, 0, f2)
        Ua1 = mk_mul(1, 0, "U", L1a, g1a1)
        Va1 = mk_mul(1, 0, "V", L1a, g2a1)
        filt_p2(0, 0, vt_l0)
        filt_p2(0, 1, vt_u0)
        filt_p2(0, 2, vt_v0)
        # img1 b-side (after Exp3)
        L1b = mk_L(nc.vector, 1, 1, f3)
        g1b1 = mk_g1(nc.vector, 1, 1, f3)
        Ub1 = mk_mul(1, 1, "U", L1b, g1b1)
        g2b1 = mk_g2(nc.gpsimd, 1, 1, f3)
        Vb1 = mk_mul(1, 1, "V", L1b, g2b1)
        fy2 = f2[:, 2]
        fy3 = f3[:, 2]
        vt_l1 = filt_p1(1, 0, fy2, fy3, "dve")
        vt_u1 = filt_p1(1, 1, rb_view(Ua1), rb_view(Ub1), "dve")
        vt_v1 = filt_p1(1, 2, rb_view(Va1), rb_view(Vb1), "act")
        filt_p2(1, 0, vt_l1)
        filt_p2(1, 1, vt_u1)
        filt_p2(1, 2, vt_v1)

        # per-channel sums of squares; l scaled by 116^2; sum channels
        nvals = float(n_ztiles * W)
        acc = None
        for ch in range(2):
            mv = acc_pool.tile([128, 2], F32, tag="mv", name="mv")
            nc.vector.bn_aggr(mv[:], stats[ch][:])
            m2 = acc_pool.tile([128, 1], F32, tag="m2", name="m2")
            nc.vector.tensor_tensor(m2[:], mv[:, 0:1], mv[:, 0:1], OP.mult)
            s = acc_pool.tile([128, 1], F32, tag=f"s{ch}", name=f"s{ch}")
            nc.vector.tensor_tensor(s[:], m2[:], mv[:, 1:2], OP.add)
            w = nvals * (116.0 * 116.0 if ch == 0 else 1.0)
            acc_new = acc_pool.tile([128, 1], F32, tag=f"acc{ch}",
                                    name=f"acc{ch}")
            if acc is None:
                nc.vector.tensor_scalar_mul(acc_new[:], s[:], w)
            else:
                nc.vector.scalar_tensor_tensor(acc_new[:], s[:], w, acc[:],
                                               OP.mult, OP.add)
            acc = acc_new
        s2 = acc_pool.tile([128, 1], F32, tag="s2", name="s2")
        nc.vector.reduce_sum(s2[:], sq[:], axis=mybir.AxisListType.X)
        acc_f = acc_pool.tile([128, 1], F32, tag="accf", name="accf")
        nc.vector.tensor_tensor(acc_f[:], s2[:], acc[:], OP.add)

        nc.sync.dma_start(out=acc_d[:], in_=acc_f[:])

    with _ActTableNarrow(nc.m.arch):
        nc.compile()
    _CACHE["nc"] = nc
    return nc


def _consts_np():
    band = np.zeros((H, H), np.float32)
    i = np.arange(H)
    for dd in range(-PAD, PAD + 1):
        j = i + dd
        m = (j >= 0) & (j < H)
        band[i[m], j[m]] = 1.0
    band = band.reshape(RB, 128, H)

    ident = np.zeros((9, 128, 128), np.float32)
    for oc in range(3):
        for ic in range(3):
            np.fill_diagonal(ident[3 * oc + ic], _M3[oc][ic])
    return (band.astype(ml_dtypes.bfloat16), (-band).astype(ml_dtypes.bfloat16),
            ident)


def _run(input, target, trace=False, **kw):
    nc = _build_nc()
    band, nband, ident = _consts_np()
    in_maps = []
    for c in range(N_CORES):
        s = slice(c * IMGS_PER_CORE, (c + 1) * IMGS_PER_CORE)
        in_maps.append({
            "inp": np.ascontiguousarray(input[s]),
            "tgt": np.ascontiguousarray(target[s]),
            "band": band,
            "nband": nband,
            "ident": ident,
        })
    return run_bass_kernel_spmd(nc, in_maps, core_ids=list(range(N_CORES)),
                                trace=trace, **kw)


def kernel(input, target, patch_size):
    assert int(np.asarray(patch_size)) == PATCH
    input = np.asarray(input, dtype=np.float32)
    target = np.asarray(target, dtype=np.float32)
    res = _run(input, target)
    total = 0.0
    for r in res.results:
        total += float(np.asarray(r["acc"]).astype(np.float64).sum())
    n = input.shape[0]
    return np.asarray(total / (n * H * W), dtype=np.float32)
